# revision 2
# baseline (speedup 1.0000x reference)
"""TAGConv GNN encoder v2: paired-row dma_gather.

The baseline is GPSIMD-bound: dma_gather descriptor generation costs ~6-10ns
per index on the Q7 ucode, and one descriptor per edge (~214k/core) is ~2ms.

v2 halves-ish the descriptor count: one descriptor fetches TWO adjacent table
rows (elem_size=512B, elem_step=256B).  The host builds a per-core permuted
gather table (with hot-source clones) ordered so that, within each
(dst-window x src-chunk) cell, most edges sit at consecutive positions.  Each
gathered slot then carries two edge "lanes" (feat cols 0:128 / 128:256); each
lane gets its own one-hot scatter matmul into the PSUM dst-window segment.

Everything else (dst-sharded SPMD over 8 cores, PSUM-chunk loop, linear +
L2-normalize tail) follows the baseline.
"""
import numpy as np
import ml_dtypes

import concourse.bass as bass
import concourse.tile as tile
from concourse import mybir, bacc
from concourse.ap import AP
from concourse.bass_utils import run_bass_kernel_spmd

F32 = mybir.dt.float32
BF16 = mybir.dt.bfloat16
I32 = mybir.dt.int32
I16 = mybir.dt.int16


def _patched_drain_and_barrier(self, tick_clock, wait_clock):
    """Tile's kernel-tail Drain carries one sync-wait per outstanding
    semaphore; the walrus build in this container can't encode more than one
    wait on one instruction. Emit each wait as its own wait_ge instead."""
    nc = self.nc
    probe = nc.sync.nop(nofuse=True)
    wait_clock.add_sem_waits(probe.ins, tile.ScopedClock({None: tick_clock.global_clock}))
    si = probe.ins.sync_info
    waits = list(si.on_wait) if si is not None else []
    if len(waits) > 1:
        si.on_wait.clear()
        sem_by_num = {h.num: h for h in self.sems.allocated().values()}
        for w in waits:
            nc.sync.wait_ge(sem_by_num[w.id], w.wait_value)
    nc.sync.drain()
    nc.all_engine_barrier()
    popped = nc._tile_sem_poison_stack.pop()
    assert popped is self._sem_poison
    nc.clear_and_free_semaphores(list(self.sems.allocated().values()))
    nc.all_engine_barrier()


tile.TileContext._drain_and_barrier = _patched_drain_and_barrier

MAX_WAITS = 1


def _split_excess_waits(nc, max_waits=MAX_WAITS):
    """Hoist sync waits beyond the per-instruction ISA budget onto NoOps."""
    for f in nc.m.functions:
        for b in f.blocks:
            ins_list = b.instructions
            out_list = []
            changed = False
            for ins in ins_list:
                si = ins.sync_info
                waits = list(si.on_wait) if si is not None else []
                if len(waits) > max_waits:
                    excess, keep = waits[:-max_waits], waits[-max_waits:]
                    for j in range(0, len(excess), max_waits):
                        nop = mybir.InstNoOp(
                            name=nc.get_next_instruction_name(), ins=[], outs=[])
                        nop.engine = ins.engine
                        nop.sync_info = mybir.SyncInfo(
                            on_wait=excess[j:j + max_waits], on_update=[])
                        out_list.append(nop)
                    ins.sync_info = mybir.SyncInfo(
                        on_wait=keep, on_update=list(si.on_update))
                    changed = True
                out_list.append(ins)
            if changed:
                b.instructions = out_list


# Problem constants
N_NODES = 100000
D = 128
HID = 128
CORES = 8

# Tuning
WIN = 256         # dst window width (one-hot / segment-matmul N)
TILE = 128        # slots per tile
CHUNK_WINS = 6    # windows per PSUM chunk (1536 f32 cols = 3 banks)
SCH = 4           # src chunks (int16 idx range)
NI_TILES = 8      # max tiles per gather instruction (8*128 = 1024 idxs)
R = 3             # rows fetched per descriptor = edge lanes per slot


def _preprocess(src, dst, n_nodes, npc, cores):
    """Host-side graph partitioning: paired-slot layout + per-core tables.

    Integer metadata + one permuted bf16 copy of h per core.
    """
    cn = n_nodes // SCH
    src = np.asarray(src).astype(np.int64)
    dst = np.asarray(dst).astype(np.int64)
    deg = np.bincount(dst, minlength=n_nodes)

    # raw edges sorted by (dst, src); src-deg norm lives in the gather table,
    # dst-deg norm is applied post-PSUM, so no per-edge scale is needed.
    o = np.lexsort((src, dst))
    usrc = src[o]
    udst = dst[o]
    core_of = udst // npc
    core_bounds = np.searchsorted(core_of, np.arange(cores + 1))

    n_wins = (npc + WIN - 1) // WIN
    ldst = udst - core_of * npc
    win = ldst // WIN
    doff = (ldst - win * WIN).astype(np.float64)
    kch = usrc // cn

    n_cells = n_wins * SCH

    percore = []
    for c in range(cores):
        sl = slice(core_bounds[c], core_bounds[c + 1])
        s_ = usrc[sl]
        w_ = win[sl]
        k_ = kch[sl]
        off_ = doff[sl]
        m = len(s_)

        # ---- clone explosion: virtual srcs with <=R window-slots ----
        o = np.lexsort((w_, s_))
        s_, w_, k_, off_ = s_[o], w_[o], k_[o], off_[o]
        first = np.r_[True, s_[1:] != s_[:-1]]
        grp_start_idx = np.flatnonzero(first)
        grp_id = np.cumsum(first) - 1
        rank = np.arange(m) - grp_start_idx[grp_id]
        lane0 = rank % R == 0
        vid = np.cumsum(lane0) - 1           # virtual index per tuple
        nv = int(vid[-1]) + 1 if m else 0
        vlane = rank % R
        vw1 = w_[lane0]
        vw2 = vw1.copy()
        vw2[vid[vlane == 1]] = w_[vlane == 1]
        vk = k_[lane0]
        vs = s_[lane0]

        # ---- table order: (chunk, w1, w2, src) ----
        vo = np.lexsort((vs, vw2, vw1, vk))
        vk_sorted = vk[vo]
        chunk_starts = np.searchsorted(vk_sorted, np.arange(SCH + 1))
        pos_sorted = np.arange(nv) - chunk_starts[vk_sorted]
        vpos = np.empty(nv, np.int64)
        vpos[vo] = pos_sorted
        rows_k = chunk_starts[1:] - chunk_starts[:-1]
        table_src = vs[vo]                    # src id per table row, chunk-major

        # ---- per-tuple position; interval cover (length R) per cell ----
        p_ = vpos[vid]
        to = np.lexsort((p_, w_, k_))
        kk, ww, pp, ooff = k_[to], w_[to], p_[to], off_[to]
        cell_id = ww * SCH + kk
        cell_first = np.r_[True, cell_id[1:] != cell_id[:-1]]
        dp = np.r_[2, pp[1:] - pp[:-1]]
        runbreak = cell_first | (dp != 1)
        run_start = np.flatnonzero(runbreak)
        run_id = np.cumsum(runbreak) - 1
        j = np.arange(m) - run_start[run_id]
        slot_lane = j % R                     # lane l reads table row p+l
        slot_in_run = j // R
        run_cell = cell_id[run_start]
        run_len = np.r_[run_start[1:], m] - run_start
        run_slots = -(-run_len // R)
        cell_slots = np.bincount(run_cell, weights=run_slots, minlength=n_cells).astype(np.int64)
        cs = np.cumsum(run_slots) - run_slots
        cell_of_run_first = np.r_[True, run_cell[1:] != run_cell[:-1]]
        cell_run_start = np.flatnonzero(cell_of_run_first)
        cell_run_id = np.cumsum(cell_of_run_first) - 1
        run_slot_base = cs - cs[cell_run_start[cell_run_id]]
        slot_in_cell = run_slot_base[run_id] + slot_in_run

        # ---- reorder slots within each cell by used-lane count (desc) ----
        # slot identity: (cell_id, slot_in_cell); lanes used = max lane + 1
        n_cs = int(cell_slots.sum())
        cell_base = np.zeros(n_cells + 1, np.int64)
        np.cumsum(cell_slots, out=cell_base[1:])
        slot_gid = cell_base[cell_id] + slot_in_cell          # per-tuple
        lanes_of_slot = np.zeros(n_cs, np.int64)
        np.maximum.at(lanes_of_slot, slot_gid, slot_lane + 1)
        cell_of_slot = np.repeat(np.arange(n_cells), cell_slots)
        so = np.lexsort((np.arange(n_cs), -lanes_of_slot, cell_of_slot))
        # rank within cell after reorder
        new_rank = np.arange(n_cs) - cell_base[cell_of_slot[so]]
        slot_new = np.empty(n_cs, np.int64)
        slot_new[so] = new_rank
        slot_in_cell = slot_new[slot_gid]

        percore.append(dict(
            m=m, cell_slots=cell_slots, rows_k=rows_k,
            lanes_of_slot=lanes_of_slot, slot_sorted=so,
            cell_of_slot=cell_of_slot, cell_base=cell_base,
            tuples=dict(kk=kk, ww=ww, pp=pp, off=ooff,
                        slot_in_cell=slot_in_cell, lane=slot_lane,
                        slot_pos=pp - slot_lane),
            table_src=table_src, chunk_starts=chunk_starts,
        ))

    # ---- uniform SPMD schedule ----
    all_cell_slots = np.stack([pc["cell_slots"] for pc in percore])  # [cores, n_cells]
    tiles_cell = -(-all_cell_slots.max(axis=0) // TILE)              # [n_cells]
    tiles_wk = tiles_cell.reshape(n_wins, SCH)
    empty = tiles_wk.sum(axis=1) == 0
    tiles_wk[empty, 0] = 1

    CROWS = int(max(pc["rows_k"].max() for pc in percore))           # uniform chunk rows
    assert CROWS + 1 < 32768

    # program tile order: psum-chunk major, then src-chunk, then window
    order = []   # (w, k)
    groups = []  # (k, t0, t1) gather instructions
    t = 0
    for p0 in range(0, n_wins, CHUNK_WINS):
        p1 = min(n_wins, p0 + CHUNK_WINS)
        for k in range(SCH):
            rt0 = t
            for w in range(p0, p1):
                order.extend([(w, k)] * int(tiles_wk[w, k]))
                t += int(tiles_wk[w, k])
            for a in range(rt0, t, NI_TILES):
                groups.append((k, a, min(t, a + NI_TILES)))
    n_tiles = t
    wk = np.array(order, np.int64) if order else np.zeros((0, 2), np.int64)
    win_of_tile = wk[:, 0]
    k_of_tile = wk[:, 1]
    # first slot (global) of each cell
    cell_tile0 = np.full(n_cells, -1, np.int64)
    for ti, (w, k) in enumerate(order):
        cid = w * SCH + k
        if cell_tile0[cid] < 0:
            cell_tile0[cid] = ti
    n_slots = n_tiles * TILE

    # ---- uniform per-tile lane counts (max over cores) ----
    lanes_tile = np.ones(n_tiles, np.int64)  # >=1 so every tile has a matmul
    slot_lanes_all = []
    for c in range(cores):
        pc = percore[c]
        tp = pc["tuples"]
        cid = tp["ww"] * SCH + tp["kk"]
        gslot = cell_tile0[cid] * TILE + tp["slot_in_cell"]
        sl_lanes = np.zeros(n_slots, np.int64)
        np.maximum.at(sl_lanes, gslot, tp["lane"] + 1)
        slot_lanes_all.append((gslot, sl_lanes))
        lanes_tile = np.maximum(lanes_tile, sl_lanes.reshape(n_tiles, TILE).max(axis=1))
    Lbase = np.zeros(n_tiles + 1, np.int64)
    np.cumsum(lanes_tile, out=Lbase[1:])
    L_total = int(Lbase[-1])

    # ---- per-core slot arrays ----
    per_core_out = []
    for c in range(cores):
        pc = percore[c]
        tp = pc["tuples"]
        gslot = slot_lanes_all[c][0]

        gidx = np.zeros(n_slots, np.int16)
        gidx[gslot] = tp["slot_pos"].astype(np.int16)
        # packed offs: [TILE, L_total]; 300 = out-of-window => dead lane
        offs_packed = np.full((TILE, L_total), 300.0, np.float32)
        tile_of = gslot // TILE
        p_of = gslot % TILE
        offs_packed[p_of, Lbase[tile_of] + tp["lane"]] = tp["off"]

        def t_(a):  # [n_slots, x] -> [128, n_tiles*x] tile-major free layout
            x = a.shape[1] if a.ndim > 1 else 1
            return np.ascontiguousarray(
                a.reshape(n_tiles, TILE, x).transpose(1, 0, 2).reshape(TILE, n_tiles * x))

        a = gidx.reshape(n_tiles, 8, 16)
        wrapped = np.transpose(a, (2, 0, 1)).reshape(16, n_tiles * 8)
        gidx16 = np.ascontiguousarray(np.tile(wrapped, (8, 1)))

        per_core_out.append(dict(
            gidx16=gidx16,
            offsP=offs_packed.astype(ml_dtypes.bfloat16),
            table_src=pc["table_src"], chunk_starts=pc["chunk_starts"],
        ))

    return dict(
        groups=groups, n_wins=n_wins, n_tiles=n_tiles,
        win_of_tile=win_of_tile, k_of_tile=k_of_tile,
        CROWS=CROWS, per_core=per_core_out,
        lanes_tile=lanes_tile, Lbase=Lbase, L_total=L_total,
    )


def _build_program(sched, n_nodes, npc):
    n_wins = sched["n_wins"]
    n_tiles = sched["n_tiles"]
    win_of_tile = sched["win_of_tile"]
    CROWS = sched["CROWS"]
    padn = n_wins * WIN
    n_chunks = -(-n_wins // CHUNK_WINS)

    lanes_tile = sched["lanes_tile"]
    Lbase = sched["Lbase"]
    L_total = sched["L_total"]
    L_instr_max = max(int(Lbase[t1] - Lbase[t0]) for (_, t0, t1) in sched["groups"])

    nc = bacc.Bacc("TRN2", target_bir_lowering=False)
    hb = nc.declare_dram_parameter("hb", [SCH * CROWS + R, D], BF16, isOutput=False)
    hself = nc.declare_dram_parameter("hself", [padn, D], BF16, isOutput=False)
    gidx_p = nc.declare_dram_parameter("gidx16", [TILE, 8 * n_tiles], I16, isOutput=False)
    offs_p = nc.declare_dram_parameter("offsP", [TILE, L_total], BF16, isOutput=False)
    norm_p = nc.declare_dram_parameter("dnorm", [1, padn], BF16, isOutput=False)
    wt_p = nc.declare_dram_parameter("wt", [2 * D, HID], BF16, isOutput=False)
    bias_p = nc.declare_dram_parameter("bias_c", [HID, 1], F32, isOutput=False)
    ident_p = nc.declare_dram_parameter("ident", [128, 128], BF16, isOutput=False)
    out_p = nc.declare_dram_parameter("out", [HID, padn], F32, isOutput=True)

    with tile.TileContext(nc) as tc:
        with (
            tc.tile_pool(name="const", bufs=1) as const,
            tc.tile_pool(name="g", bufs=3) as gpool,
            tc.tile_pool(name="oh", bufs=3) as ohpool,
            tc.tile_pool(name="hr", bufs=2) as hrpool,
            tc.tile_pool(name="slab", bufs=2) as slab,
            tc.tile_pool(name="y", bufs=6) as ypool,
            tc.tile_pool(name="aggps", bufs=1, space="PSUM") as agg_ps,
            tc.tile_pool(name="scrps", bufs=4, space="PSUM") as scr_ps,
        ):
            gidx_sb = const.tile([TILE, 8 * n_tiles], I16)
            nc.sync.dma_start(gidx_sb[:], gidx_p[:])
            offs_sb = const.tile([TILE, L_total], BF16)
            nc.sync.dma_start(offs_sb[:], offs_p[:])
            norm_sb = const.tile([1, padn], BF16)
            nc.sync.dma_start(norm_sb[:], norm_p[:])
            ones_k1 = const.tile([1, 128], BF16)
            nc.vector.memset(ones_k1[:], 1.0)

            w1_sb = const.tile([D, HID], BF16)
            nc.sync.dma_start(w1_sb[:], wt_p[0:D, :])
            w2_sb = const.tile([D, HID], BF16)
            nc.sync.dma_start(w2_sb[:], wt_p[D:2 * D, :])
            bias_sb = const.tile([HID, 1], F32)
            nc.sync.dma_start(bias_sb[:], bias_p[:])
            ident_sb = const.tile([128, 128], BF16)
            nc.sync.dma_start(ident_sb[:], ident_p[:])
            ones_sb = const.tile([128, 128], F32)
            nc.vector.memset(ones_sb[:], 1.0)
            iota_i = const.tile([128, WIN], I32)
            nc.gpsimd.iota(iota_i[:], pattern=[[1, WIN]], base=0, channel_multiplier=0)
            iota_b = const.tile([128, WIN], BF16)
            nc.vector.tensor_copy(iota_b[:], iota_i[:])

            ni_regs = {}

            def ni_reg(n):
                if n not in ni_regs:
                    r = nc.gpsimd.alloc_register()
                    nc.gpsimd.reg_mov(r, n)
                    ni_regs[n] = r
                return ni_regs[n]

            groups = sched["groups"]
            group_by_start = {g[1]: g for g in groups}

            tile_of_chunk = [[] for _ in range(n_chunks)]
            for t in range(n_tiles):
                tile_of_chunk[int(win_of_tile[t]) // CHUNK_WINS].append(t)

            for ch in range(n_chunks):
                w0 = ch * CHUNK_WINS
                w1 = min(n_wins, w0 + CHUNK_WINS)
                cw = (w1 - w0) * WIN
                col0 = w0 * WIN
                tlist = tile_of_chunk[ch]
                assert tlist == list(range(tlist[0], tlist[-1] + 1))
                t0c, t1c = tlist[0], tlist[-1] + 1

                bank_of = [(int(win_of_tile[t]) - w0) * WIN * 4 // 2048 for t in tlist]
                first_of_bank, last_of_bank = {}, {}
                for t, bk in zip(tlist, bank_of):
                    first_of_bank.setdefault(bk, t)
                    last_of_bank[bk] = t

                pagg = agg_ps.tile([128, CHUNK_WINS * WIN], F32, tag="pagg")

                g0 = t0c
                while g0 < t1c:
                    k, ta, gend = group_by_start[g0]
                    assert ta == g0
                    gt = gend - g0
                    Lb0 = int(Lbase[g0])
                    Li = int(Lbase[gend]) - Lb0
                    G = gpool.tile([128, NI_TILES, R * D], BF16, tag="G")
                    in_ap = AP(hb[:, :].tensor, k * CROWS * D,
                               [[D, CROWS + 1], [1, R * D]])
                    nc.gpsimd.dma_gather(
                        out_ap=G[:, :gt, :],
                        in_ap=in_ap,
                        idxs_ap=gidx_sb[:, 8 * g0:8 * gend],
                        num_idxs=TILE * gt,
                        num_idxs_reg=ni_reg(TILE * gt),
                        elem_size=R * D,
                        elem_step=D,
                    )
                    # packed one-hots: only the lanes tiles actually use
                    oh = ohpool.tile([128, L_instr_max, WIN], BF16, tag="oh")
                    off_bc = (offs_sb[:, Lb0:Lb0 + Li]
                              .unsqueeze(2).broadcast_to([128, Li, WIN]))
                    iota_bc = iota_b[:].unsqueeze(1).broadcast_to([128, Li, WIN])
                    nc.vector.tensor_tensor(out=oh[:, :Li, :], in0=off_bc,
                                            in1=iota_bc, op=mybir.AluOpType.is_equal)
                    for x in range(gt):
                        t = g0 + x
                        col = (int(win_of_tile[t]) - w0) * WIN
                        bk = bank_of[t - t0c]
                        nl = int(lanes_tile[t])
                        for l in range(nl):
                            nc.tensor.matmul(
                                pagg[:, col:col + WIN],
                                lhsT=G[:, x, l * D:(l + 1) * D],
                                rhs=oh[:, int(Lbase[t]) - Lb0 + l, :],
                                start=(first_of_bank[bk] == t and l == 0),
                                stop=(last_of_bank[bk] == t and l == nl - 1),
                                skip_group_check=True,
                            )
                    g0 = gend

                # materialize the dst-degree norm row across partitions (PE
                # outer product), then fold it in while evacuating PSUM
                normf = slab.tile([128, CHUNK_WINS * WIN], F32, tag="normf")
                for bs in range(0, cw, 512):
                    bw = min(512, cw - bs)
                    pn = scr_ps.tile([128, 512], F32, tag="scr")
                    nc.tensor.matmul(pn[:, :bw], lhsT=ones_k1[:],
                                     rhs=norm_sb[0:1, col0 + bs:col0 + bs + bw],
                                     start=True, stop=True)
                    nc.vector.tensor_copy(normf[:, bs:bs + bw], pn[:, :bw])
                aggT = slab.tile([128, CHUNK_WINS * WIN], BF16, tag="aggT")
                nc.vector.tensor_tensor(out=aggT[:, :cw], in0=pagg[:, :cw],
                                        in1=normf[:, :cw], op=mybir.AluOpType.mult)

                nh = cw // 128
                hr = hrpool.tile([128, CHUNK_WINS * WIN // 128, D], BF16, tag="hr")
                nc.sync.dma_start(
                    hr[:, :nh, :],
                    hself[col0:col0 + cw, :].rearrange("(x p) f -> p x f", p=128),
                )
                hT = slab.tile([128, CHUNK_WINS * WIN], BF16, tag="hT")
                for xt in range(nh):
                    pt = scr_ps.tile([128, 128], BF16, tag="scr")
                    nc.tensor.transpose(pt[:], hr[:, xt, :], ident_sb[:])
                    nc.vector.tensor_copy(hT[:, xt * 128:(xt + 1) * 128], pt[:])

                BANK = 512
                for bs in range(0, cw, BANK):
                    bw = min(BANK, cw - bs)
                    po = scr_ps.tile([128, BANK], F32, tag="scr")
                    nc.tensor.matmul(po[:, :bw], lhsT=w1_sb[:], rhs=hT[:, bs:bs + bw],
                                     start=True, stop=False)
                    nc.tensor.matmul(po[:, :bw], lhsT=w2_sb[:], rhs=aggT[:, bs:bs + bw],
                                     start=False, stop=True)
                    y = ypool.tile([128, BANK], F32, tag="y")
                    nc.scalar.activation(y[:, :bw], po[:, :bw],
                                         mybir.ActivationFunctionType.Identity,
                                         bias=bias_sb[:])
                    z = ypool.tile([128, BANK], F32, tag="z")
                    nc.scalar.square(z[:, :bw], y[:, :bw])
                    pr = scr_ps.tile([128, BANK], F32, tag="scr")
                    nc.tensor.matmul(pr[:, :bw], lhsT=ones_sb[:], rhs=z[:, :bw],
                                     start=True, stop=True)
                    rs = ypool.tile([128, BANK], F32, tag="rs")
                    nc.vector.reciprocal(rs[:, :bw], pr[:, :bw])
                    nc.scalar.sqrt(rs[:, :bw], rs[:, :bw])
                    of = ypool.tile([128, BANK], F32, tag="of")
                    nc.vector.tensor_tensor(out=of[:, :bw], in0=y[:, :bw],
                                            in1=rs[:, :bw], op=mybir.AluOpType.mult)
                    nc.sync.dma_start(out_p[:, col0 + bs:col0 + bs + bw], of[:, :bw])

    nc.finalize()
    _split_excess_waits(nc)
    return nc


def _run(h, weight, bias, src, dst, n_nodes, npc, cores, trace=False):
    sched = _preprocess(src, dst, n_nodes, npc, cores)
    nc = _build_program(sched, n_nodes, npc)

    padn = sched["n_wins"] * WIN
    CROWS = sched["CROWS"]
    h = np.asarray(h, dtype=np.float32)
    hb_all = h.astype(ml_dtypes.bfloat16)
    deg = np.bincount(np.asarray(dst).astype(np.int64), minlength=n_nodes)
    dn = (1.0 / np.sqrt(np.clip(deg, 1, None))).astype(np.float32)
    fh_all = (h * dn[:, None]).astype(ml_dtypes.bfloat16)  # src-norm in table
    wt = np.asarray(weight, dtype=np.float32).astype(ml_dtypes.bfloat16)
    bias_c = np.ascontiguousarray(np.asarray(bias, dtype=np.float32).reshape(HID, 1))
    ident = np.eye(128, dtype=np.float32).astype(ml_dtypes.bfloat16)

    in_maps = []
    for c in range(cores):
        pc = sched["per_core"][c]
        table = np.zeros((SCH * CROWS + R, D), dtype=ml_dtypes.bfloat16)
        cs = pc["chunk_starts"]
        for k in range(SCH):
            rows = pc["table_src"][cs[k]:cs[k + 1]]
            table[k * CROWS:k * CROWS + len(rows)] = fh_all[rows]
        hself = np.zeros((padn, D), dtype=ml_dtypes.bfloat16)
        hself[:npc] = hb_all[c * npc:(c + 1) * npc]
        dnorm = np.zeros((1, padn), dtype=ml_dtypes.bfloat16)
        dnorm[0, :npc] = dn[c * npc:(c + 1) * npc]
        in_maps.append(dict(
            hb=table, hself=hself, dnorm=dnorm,
            gidx16=pc["gidx16"], offsP=pc["offsP"],
            wt=wt, bias_c=bias_c, ident=ident,
        ))

    res = run_bass_kernel_spmd(nc, in_maps, core_ids=list(range(cores)), trace=trace)
    out = np.empty((cores * npc, HID), dtype=np.float32)
    for c in range(cores):
        out[c * npc:(c + 1) * npc] = res.results[c]["out"][:, :npc].T
    return out, res


def kernel(h, weight, bias, src, dst):
    out, _ = _run(h, weight, bias, src, dst, N_NODES, N_NODES // CORES, CORES)
    return out
